# revision 13
# baseline (speedup 1.0000x reference)
"""Binarized 3x3 conv (sign(W) conv + bias) on 8 Trainium2 NeuronCores.

Problem (hardcoded):
  x:      (32, 256, 56, 56) f32
  weight: (256, 256, 3, 3)  f32  -> sign-binarized
  bias:   (256,)            f32
  out:    (32, 256, 56, 56) f32  (stride 1, pad 1)

Sharding/marshaling (host): data-parallel over batch — 4 images per
core, weight/bias replicated. The weight is passed to the device
pre-permuted to [IC, OC, 3, 3] (a pure layout transform, like the batch
slicing) so that the matmul lhsT tiles come out of a single on-device
sign() with no transposes; all arithmetic (sign, conv, bias) runs on
device.

Per-core kernel: conv = sum over the 9 taps of shifted 1x1 convs. x is
cast to bf16 (DVE) into a zero-padded [128, 2, 58, 58] SBUF image in
row slabs so matmuls can start before the whole image has landed.
Weights are sign-binarized per (ic-chunk, oc-chunk) on ACT (Sign: 0->0,
matching jnp.sign) directly into tap-major lhsT tiles [ic, oc] (bf16 is
exact for ±1/0). Each output tile [oc=128, 8 rows x 56 cols = 448]
accumulates 2 ic-chunks x 9 taps = 18 bf16 matmuls in one PSUM bank
(fp32 accumulate), then gets the per-channel bias added on DVE and is
DMA'd out.
"""

import numpy as np

import concourse.bass as bass
import concourse.mybir as mybir
import concourse.tile as tile
from concourse import bacc
from concourse.bass_utils import run_bass_kernel_spmd

N_CORES = 8
B = 32
B_PER = B // N_CORES  # 4 images per core
IC = OC = 256
H = W = 56
K = 3
R = 8               # output rows per matmul group
G = H // R          # 7 row groups
NCH = IC // 128     # 2 ic chunks
OCH = OC // 128     # 2 oc chunks
SLAB = 7            # x load/cast row-slab height
NSLAB = H // SLAB   # 8 slabs

# Results of the last run_bass_kernel_spmd call (exec_time_ns etc.) for
# introspection by test harnesses; not used for grading.
LAST_RESULTS = None

_CACHED_NC = None


def _build_nc() -> bass.Bass:
    nc = bacc.Bacc()
    x = nc.dram_tensor("x", [B_PER, IC, H, W], mybir.dt.float32,
                       kind="ExternalInput")
    wt = nc.dram_tensor("weight_t", [IC, OC, K, K], mybir.dt.float32,
                        kind="ExternalInput")
    bs = nc.dram_tensor("bias", [OC], mybir.dt.float32, kind="ExternalInput")
    out = nc.dram_tensor("out", [B_PER, OC, H, W], mybir.dt.float32,
                         kind="ExternalOutput")

    with tile.TileContext(nc) as tc:
        with (
            tc.tile_pool(name="const", bufs=1) as const_pool,
            tc.tile_pool(name="wprep", bufs=2) as wprep_pool,
            tc.tile_pool(name="xs", bufs=6) as xs_pool,
            tc.tile_pool(name="xp", bufs=2) as xp_pool,
            tc.tile_pool(name="osb", bufs=4) as out_pool,
            tc.tile_pool(name="psum", bufs=2, space="PSUM") as psum_pool,
        ):
            bias_sb = const_pool.tile([128, OCH], mybir.dt.float32, tag="bias")
            nc.sync.dma_start(bias_sb, bs.rearrange("(a p) -> p a", p=128))

            # ---- weight prep: sign-binarize into lhsT tap tiles ----
            # w_taps[ic_part, c, o, t, oc] : lhsT for (ic chunk c, oc chunk o, tap t)
            w_taps = const_pool.tile([128, NCH, OCH, K * K, 128],
                                     mybir.dt.bfloat16, tag="wtaps")

            def prep_weights(o):
                for c in range(NCH):
                    wf = wprep_pool.tile([128, 128, K * K], mybir.dt.float32,
                                         tag="wf", bufs=2, name=f"wf_{o}_{c}")
                    # split over oc so the load spreads across 4 DMA queues
                    # (one dma_start lands on a single ~60 GB/s queue)
                    for j in range(4):
                        nc.sync.dma_start(
                            wf[:, j * 32:(j + 1) * 32, :],
                            wt[c * 128:(c + 1) * 128,
                               o * 128 + j * 32:o * 128 + (j + 1) * 32]
                            .rearrange("i o a b -> i o (a b)"))
                    # [ic, oc, t] -> tap-major [ic, t, oc], sign, cast bf16
                    nc.scalar.sign(w_taps[:, c, o, :, :],
                                   wf.rearrange("p o t -> p t o"))

            def load_slab(n, xpd, s):
                for c in range(NCH):
                    xs = xs_pool.tile([128, SLAB, W], mybir.dt.float32,
                                      tag="xs", name=f"xs_{n}_{s}_{c}")
                    nc.sync.dma_start(
                        xs, x[n, c * 128:(c + 1) * 128,
                              s * SLAB:(s + 1) * SLAB, :])
                    nc.vector.tensor_copy(
                        out=xpd[:, c, 1 + s * SLAB:1 + (s + 1) * SLAB,
                                1:W + 1],
                        in_=xs)

            # o=0 weights first, then image-0's first slab, then o=1 weights:
            # keeps the early DMA queues clear for the first-matmul deps.
            prep_weights(0)

            # ---- per-image conv ----
            for n in range(B_PER):
                # zero-padded bf16 image [128, c, 58, 58]
                xpd = xp_pool.tile([128, NCH, H + 2, W + 2], mybir.dt.bfloat16,
                                   tag="xpd", name=f"xpd_{n}")
                for c in range(NCH):
                    nc.any.memset(xpd[:, c, 0, :], 0.0)
                    nc.any.memset(xpd[:, c, H + 1, :], 0.0)
                    nc.any.memset(xpd[:, c, 1:H + 1, 0], 0.0)
                    nc.any.memset(xpd[:, c, 1:H + 1, W + 1], 0.0)
                # load + cast in row slabs so matmuls can start early
                for s in range(NSLAB):
                    load_slab(n, xpd, s)
                    if n == 0 and s == 0:
                        prep_weights(1)

                for o in range(OCH):
                    for g in range(G):
                        ps = psum_pool.tile([128, R, W], mybir.dt.float32,
                                            tag="acc", bufs=8)
                        for c in range(NCH):
                            for ky in range(K):
                                for kx in range(K):
                                    t = ky * K + kx
                                    nc.tensor.matmul(
                                        ps,
                                        w_taps[:, c, o, t, :],
                                        xpd[:, c, g * R + ky:g * R + ky + R,
                                            kx:kx + W],
                                        start=(c == 0 and t == 0),
                                        stop=(c == NCH - 1 and t == K * K - 1),
                                    )
                        osb = out_pool.tile([128, R, W], mybir.dt.float32,
                                            tag="osb")
                        nc.vector.tensor_tensor(
                            osb, ps,
                            bias_sb[:, o:o + 1, None].to_broadcast((128, R, W)),
                            mybir.AluOpType.add)
                        nc.sync.dma_start(
                            out[n, o * 128:(o + 1) * 128,
                                g * R:(g + 1) * R, :],
                            osb)
    nc.finalize()
    return nc


def kernel(x: np.ndarray, weight: np.ndarray, bias: np.ndarray) -> np.ndarray:
    global LAST_RESULTS, _CACHED_NC
    assert x.shape == (B, IC, H, W)
    if _CACHED_NC is None:
        _CACHED_NC = _build_nc()
    nc = _CACHED_NC

    # pure layout transform: [OC, IC, 3, 3] -> [IC, OC, 3, 3]
    weight_t = np.ascontiguousarray(
        np.asarray(weight, dtype=np.float32).transpose(1, 0, 2, 3))
    bias = np.ascontiguousarray(bias, dtype=np.float32)
    in_maps = [
        {
            "x": np.ascontiguousarray(x[i * B_PER:(i + 1) * B_PER],
                                      dtype=np.float32),
            "weight_t": weight_t,
            "bias": bias,
        }
        for i in range(N_CORES)
    ]
    res = run_bass_kernel_spmd(nc, in_maps, core_ids=list(range(N_CORES)))
    LAST_RESULTS = res
    return np.concatenate([res.results[i]["out"] for i in range(N_CORES)],
                          axis=0)


# revision 15
# speedup vs baseline: 1.0281x; 1.0281x over previous
"""Binarized 3x3 conv (sign(W) conv + bias) on 8 Trainium2 NeuronCores.

Problem (hardcoded):
  x:      (32, 256, 56, 56) f32
  weight: (256, 256, 3, 3)  f32  -> sign-binarized
  bias:   (256,)            f32
  out:    (32, 256, 56, 56) f32  (stride 1, pad 1)

Sharding/marshaling (host): data-parallel over batch — 4 images per
core, weight/bias replicated. The weight is passed to the device
pre-permuted to [IC, OC, 3, 3] (a pure layout transform, like the batch
slicing) so that the matmul lhsT tiles come out of a single on-device
sign() with no transposes; all arithmetic (sign, conv, bias) runs on
device.

Per-core kernel: conv = sum over the 9 taps of shifted 1x1 convs. x is
cast to bf16 (DVE) into a zero-padded [128, 2, 58, 58] SBUF image in
row slabs so matmuls can start before the whole image has landed.
Weights are sign-binarized per (ic-chunk, oc-chunk) on ACT (Sign: 0->0,
matching jnp.sign) directly into tap-major lhsT tiles [ic, oc] (bf16 is
exact for ±1/0). Each output tile [oc=128, 8 rows x 56 cols = 448]
accumulates 2 ic-chunks x 9 taps = 18 bf16 matmuls in one PSUM bank
(fp32 accumulate), then gets the per-channel bias added on DVE and is
DMA'd out.
"""

import numpy as np

import concourse.bass as bass
import concourse.mybir as mybir
import concourse.tile as tile
from concourse import bacc
from concourse.bass_utils import run_bass_kernel_spmd

N_CORES = 8
B = 32
B_PER = B // N_CORES  # 4 images per core
IC = OC = 256
H = W = 56
K = 3
R = 8               # output rows per matmul group
G = H // R          # 7 row groups
NCH = IC // 128     # 2 ic chunks
OCH = OC // 128     # 2 oc chunks
SLAB = 14           # x load/cast row-slab height
NSLAB = H // SLAB   # 4 slabs

# Results of the last run_bass_kernel_spmd call (exec_time_ns etc.) for
# introspection by test harnesses; not used for grading.
LAST_RESULTS = None

_CACHED_NC = None


def _build_nc() -> bass.Bass:
    nc = bacc.Bacc()
    x = nc.dram_tensor("x", [B_PER, IC, H, W], mybir.dt.float32,
                       kind="ExternalInput")
    wt = nc.dram_tensor("weight_t", [IC, OC, K, K], mybir.dt.float32,
                        kind="ExternalInput")
    bs = nc.dram_tensor("bias", [OC], mybir.dt.float32, kind="ExternalInput")
    out = nc.dram_tensor("out", [B_PER, OC, H, W], mybir.dt.float32,
                         kind="ExternalOutput")

    with tile.TileContext(nc) as tc:
        with (
            tc.tile_pool(name="const", bufs=1) as const_pool,
            tc.tile_pool(name="wprep", bufs=2) as wprep_pool,
            tc.tile_pool(name="xs", bufs=6) as xs_pool,
            tc.tile_pool(name="xp", bufs=2) as xp_pool,
            tc.tile_pool(name="osb", bufs=4) as out_pool,
            tc.tile_pool(name="psum", bufs=2, space="PSUM") as psum_pool,
        ):
            bias_sb = const_pool.tile([128, OCH], mybir.dt.float32, tag="bias")
            nc.sync.dma_start(bias_sb, bs.rearrange("(a p) -> p a", p=128))

            # ---- weight prep: sign-binarize into lhsT tap tiles ----
            # w_taps[ic_part, c, o, t, oc] : lhsT for (ic chunk c, oc chunk o, tap t)
            w_taps = const_pool.tile([128, NCH, OCH, K * K, 128],
                                     mybir.dt.bfloat16, tag="wtaps")

            def prep_weights(o):
                for c in range(NCH):
                    wf = wprep_pool.tile([128, 128, K * K], mybir.dt.float32,
                                         tag="wf", bufs=2, name=f"wf_{o}_{c}")
                    # issue from the scalar engine's HWDGE: its queues are
                    # idle early, so the weight path doesn't contend with
                    # the x slab loads; split over oc for queue parallelism
                    for j in range(4):
                        nc.scalar.dma_start(
                            wf[:, j * 32:(j + 1) * 32, :],
                            wt[c * 128:(c + 1) * 128,
                               o * 128 + j * 32:o * 128 + (j + 1) * 32]
                            .rearrange("i o a b -> i o (a b)"))
                    # [ic, oc, t] -> tap-major [ic, t, oc], sign, cast bf16
                    nc.scalar.sign(w_taps[:, c, o, :, :],
                                   wf.rearrange("p o t -> p t o"))

            def load_slab(n, xpd, s):
                for c in range(NCH):
                    xs = xs_pool.tile([128, SLAB, W], mybir.dt.float32,
                                      tag="xs", name=f"xs_{n}_{s}_{c}")
                    nc.sync.dma_start(
                        xs, x[n, c * 128:(c + 1) * 128,
                              s * SLAB:(s + 1) * SLAB, :])
                    nc.vector.tensor_copy(
                        out=xpd[:, c, 1 + s * SLAB:1 + (s + 1) * SLAB,
                                1:W + 1],
                        in_=xs)

            # o=0 weights first, then image-0's first slab, then o=1 weights:
            # keeps the early DMA queues clear for the first-matmul deps.
            prep_weights(0)

            # ---- per-image conv ----
            for n in range(B_PER):
                # zero-padded bf16 image [128, c, 58, 58]
                xpd = xp_pool.tile([128, NCH, H + 2, W + 2], mybir.dt.bfloat16,
                                   tag="xpd", name=f"xpd_{n}")
                for c in range(NCH):
                    nc.any.memset(xpd[:, c, 0, :], 0.0)
                    nc.any.memset(xpd[:, c, H + 1, :], 0.0)
                    nc.any.memset(xpd[:, c, 1:H + 1, 0], 0.0)
                    nc.any.memset(xpd[:, c, 1:H + 1, W + 1], 0.0)
                # load + cast in row slabs so matmuls can start early
                for s in range(NSLAB):
                    load_slab(n, xpd, s)
                    if n == 0 and s == 0:
                        prep_weights(1)

                for o in range(OCH):
                    for g in range(G):
                        ps = psum_pool.tile([128, R, W], mybir.dt.float32,
                                            tag="acc", bufs=8)
                        for c in range(NCH):
                            for ky in range(K):
                                for kx in range(K):
                                    t = ky * K + kx
                                    nc.tensor.matmul(
                                        ps,
                                        w_taps[:, c, o, t, :],
                                        xpd[:, c, g * R + ky:g * R + ky + R,
                                            kx:kx + W],
                                        start=(c == 0 and t == 0),
                                        stop=(c == NCH - 1 and t == K * K - 1),
                                    )
                        osb = out_pool.tile([128, R, W], mybir.dt.float32,
                                            tag="osb")
                        nc.vector.tensor_tensor(
                            osb, ps,
                            bias_sb[:, o:o + 1, None].to_broadcast((128, R, W)),
                            mybir.AluOpType.add)
                        nc.sync.dma_start(
                            out[n, o * 128:(o + 1) * 128,
                                g * R:(g + 1) * R, :],
                            osb)
    nc.finalize()
    return nc


def kernel(x: np.ndarray, weight: np.ndarray, bias: np.ndarray) -> np.ndarray:
    global LAST_RESULTS, _CACHED_NC
    assert x.shape == (B, IC, H, W)
    if _CACHED_NC is None:
        _CACHED_NC = _build_nc()
    nc = _CACHED_NC

    # pure layout transform: [OC, IC, 3, 3] -> [IC, OC, 3, 3]
    weight_t = np.ascontiguousarray(
        np.asarray(weight, dtype=np.float32).transpose(1, 0, 2, 3))
    bias = np.ascontiguousarray(bias, dtype=np.float32)
    in_maps = [
        {
            "x": np.ascontiguousarray(x[i * B_PER:(i + 1) * B_PER],
                                      dtype=np.float32),
            "weight_t": weight_t,
            "bias": bias,
        }
        for i in range(N_CORES)
    ]
    res = run_bass_kernel_spmd(nc, in_maps, core_ids=list(range(N_CORES)))
    LAST_RESULTS = res
    return np.concatenate([res.results[i]["out"] for i in range(N_CORES)],
                          axis=0)


# revision 17
# speedup vs baseline: 1.0309x; 1.0027x over previous
"""Binarized 3x3 conv (sign(W) conv + bias) on 8 Trainium2 NeuronCores.

Problem (hardcoded):
  x:      (32, 256, 56, 56) f32
  weight: (256, 256, 3, 3)  f32  -> sign-binarized
  bias:   (256,)            f32
  out:    (32, 256, 56, 56) f32  (stride 1, pad 1)

Sharding/marshaling (host): data-parallel over batch — 4 images per
core, weight/bias replicated. The weight is passed to the device
pre-permuted to [IC, OC, 3, 3] (a pure layout transform, like the batch
slicing) so that the matmul lhsT tiles come out of a single on-device
sign() with no transposes; all arithmetic (sign, conv, bias) runs on
device.

Per-core kernel: conv = sum over the 9 taps of shifted 1x1 convs. x is
cast to bf16 (DVE) into a zero-padded [128, 2, 58, 58] SBUF image in
row slabs so matmuls can start before the whole image has landed.
Weights are sign-binarized per (ic-chunk, oc-chunk) on ACT (Sign: 0->0,
matching jnp.sign) directly into tap-major lhsT tiles [ic, oc] (bf16 is
exact for ±1/0). Each output tile [oc=128, 8 rows x 56 cols = 448]
accumulates 2 ic-chunks x 9 taps = 18 bf16 matmuls in one PSUM bank
(fp32 accumulate), then gets the per-channel bias added on DVE and is
DMA'd out.
"""

import numpy as np

import concourse.bass as bass
import concourse.mybir as mybir
import concourse.tile as tile
from concourse import bacc
from concourse.bass_utils import run_bass_kernel_spmd

N_CORES = 8
B = 32
B_PER = B // N_CORES  # 4 images per core
IC = OC = 256
H = W = 56
K = 3
R = 8               # output rows per matmul group
G = H // R          # 7 row groups
NCH = IC // 128     # 2 ic chunks
OCH = OC // 128     # 2 oc chunks
SLAB = 14           # x load/cast row-slab height
NSLAB = H // SLAB   # 4 slabs

# Results of the last run_bass_kernel_spmd call (exec_time_ns etc.) for
# introspection by test harnesses; not used for grading.
LAST_RESULTS = None

_CACHED_NC = None


def _build_nc() -> bass.Bass:
    nc = bacc.Bacc()
    x = nc.dram_tensor("x", [B_PER, IC, H, W], mybir.dt.float32,
                       kind="ExternalInput")
    wt = nc.dram_tensor("weight_t", [IC, OC, K, K], mybir.dt.float32,
                        kind="ExternalInput")
    bs = nc.dram_tensor("bias", [OC], mybir.dt.float32, kind="ExternalInput")
    out = nc.dram_tensor("out", [B_PER, OC, H, W], mybir.dt.float32,
                         kind="ExternalOutput")

    with tile.TileContext(nc) as tc:
        with (
            tc.tile_pool(name="const", bufs=1) as const_pool,
            tc.tile_pool(name="wprep", bufs=2) as wprep_pool,
            tc.tile_pool(name="xs", bufs=6) as xs_pool,
            tc.tile_pool(name="xp", bufs=2) as xp_pool,
            tc.tile_pool(name="osb", bufs=4) as out_pool,
            tc.tile_pool(name="psum", bufs=2, space="PSUM") as psum_pool,
        ):
            # ---- weight prep: sign-binarize into lhsT tap tiles ----
            # w_taps[ic_part, c, o, t, oc] : lhsT for (ic chunk c, oc chunk o, tap t)
            w_taps = const_pool.tile([128, NCH, OCH, K * K, 128],
                                     mybir.dt.bfloat16, tag="wtaps")

            def prep_weights(o):
                for c in range(NCH):
                    wf = wprep_pool.tile([128, 128, K * K], mybir.dt.float32,
                                         tag="wf", bufs=2, name=f"wf_{o}_{c}")
                    nc.sync.dma_start(
                        wf,
                        wt[c * 128:(c + 1) * 128, o * 128:(o + 1) * 128]
                        .rearrange("i o a b -> i o (a b)"))
                    # per-tap sign: the first matmul only needs tap 0, so
                    # don't gate it on the full [128,1152] sign
                    for t in range(K * K):
                        nc.scalar.sign(w_taps[:, c, o, t, :], wf[:, :, t])

            prep_weights(0)
            bias_sb = const_pool.tile([128, OCH], mybir.dt.float32, tag="bias")
            nc.sync.dma_start(bias_sb, bs.rearrange("(a p) -> p a", p=128))

            def load_slab(n, xpd, s):
                for c in range(NCH):
                    xs = xs_pool.tile([128, SLAB, W], mybir.dt.float32,
                                      tag="xs", name=f"xs_{n}_{s}_{c}")
                    nc.sync.dma_start(
                        xs, x[n, c * 128:(c + 1) * 128,
                              s * SLAB:(s + 1) * SLAB, :])
                    nc.vector.tensor_copy(
                        out=xpd[:, c, 1 + s * SLAB:1 + (s + 1) * SLAB,
                                1:W + 1],
                        in_=xs)

            # ---- per-image conv ----
            for n in range(B_PER):
                # zero-padded bf16 image [128, c, 58, 58]
                xpd = xp_pool.tile([128, NCH, H + 2, W + 2], mybir.dt.bfloat16,
                                   tag="xpd", name=f"xpd_{n}")
                for c in range(NCH):
                    nc.any.memset(xpd[:, c, 0, :], 0.0)
                    nc.any.memset(xpd[:, c, H + 1, :], 0.0)
                    nc.any.memset(xpd[:, c, 1:H + 1, 0], 0.0)
                    nc.any.memset(xpd[:, c, 1:H + 1, W + 1], 0.0)
                # load + cast in row slabs so matmuls can start early
                for s in range(NSLAB):
                    load_slab(n, xpd, s)
                    if n == 0 and s == 0:
                        prep_weights(1)

                for o in range(OCH):
                    for g in range(G):
                        ps = psum_pool.tile([128, R, W], mybir.dt.float32,
                                            tag="acc", bufs=8)
                        for c in range(NCH):
                            for ky in range(K):
                                for kx in range(K):
                                    t = ky * K + kx
                                    nc.tensor.matmul(
                                        ps,
                                        w_taps[:, c, o, t, :],
                                        xpd[:, c, g * R + ky:g * R + ky + R,
                                            kx:kx + W],
                                        start=(c == 0 and t == 0),
                                        stop=(c == NCH - 1 and t == K * K - 1),
                                    )
                        osb = out_pool.tile([128, R, W], mybir.dt.float32,
                                            tag="osb")
                        nc.vector.tensor_tensor(
                            osb, ps,
                            bias_sb[:, o:o + 1, None].to_broadcast((128, R, W)),
                            mybir.AluOpType.add)
                        nc.sync.dma_start(
                            out[n, o * 128:(o + 1) * 128,
                                g * R:(g + 1) * R, :],
                            osb)
    nc.finalize()
    return nc


def kernel(x: np.ndarray, weight: np.ndarray, bias: np.ndarray) -> np.ndarray:
    global LAST_RESULTS, _CACHED_NC
    assert x.shape == (B, IC, H, W)
    if _CACHED_NC is None:
        _CACHED_NC = _build_nc()
    nc = _CACHED_NC

    # pure layout transform: [OC, IC, 3, 3] -> [IC, OC, 3, 3]
    weight_t = np.ascontiguousarray(
        np.asarray(weight, dtype=np.float32).transpose(1, 0, 2, 3))
    bias = np.ascontiguousarray(bias, dtype=np.float32)
    in_maps = [
        {
            "x": np.ascontiguousarray(x[i * B_PER:(i + 1) * B_PER],
                                      dtype=np.float32),
            "weight_t": weight_t,
            "bias": bias,
        }
        for i in range(N_CORES)
    ]
    res = run_bass_kernel_spmd(nc, in_maps, core_ids=list(range(N_CORES)))
    LAST_RESULTS = res
    return np.concatenate([res.results[i]["out"] for i in range(N_CORES)],
                          axis=0)
